# revision 34
# baseline (speedup 1.0000x reference)
"""Trainium2 Bass kernel: DeepseekV4 CSA Compressor.

Math (per batch b):
  kv = hidden @ w_kv, gate = hidden @ w_gate          [S, 256]
  windows w = 0..S/32-1: tokens [w*32-32, w*32+32)  (prev block -> lo
  channels, current block -> hi channels; window 0 prev = 0 kv / -1e9 gate)
  pooled[w] = sum_j softmax_j(win_g + pos_bias)[j, d] * win_kv[j, d]
  RoPE on trailing 64 dims at position w*32.

Sharding: 8 cores = (4 batches) x (2 sequence halves).  Each core gets its
4096-token chunk transposed on host ([H, 4128] with a 32-token halo column
block in front; zeros for the first half, so the -1e9 gate fill is applied
via a per-core bias variant on the first window group).  No collectives.

Matmuls run in bfloat16 (1 col/cycle on the PE, same as f32r, but half
the HBM/DMA traffic).  Inputs are converted to bf16 on host; measured
end-to-end rel err ~2.4e-3 vs the fp32 reference (threshold 2e-2).
"""

import ml_dtypes
import numpy as np

BF16 = ml_dtypes.bfloat16

HEAD_DIM = 128
ROPE_DIM = 64
RATIO = 32
ROPE_THETA = 10000.0
NEG = -1e9

B, S, H = 4, 8192, 4096
N_CORES = 8
HALF = S // 2                 # tokens per core
NWIN_CORE = HALF // RATIO     # windows per core = 128
GW = 512                      # tokens per matmul/pooling group
WPG = GW // RATIO             # windows per group = 16

_CACHE: dict = {}


def _round_f32r(x: np.ndarray) -> np.ndarray:
    """Round fp32 to f32r (11-bit mantissa, round-to-nearest-even)."""
    u = np.ascontiguousarray(x, dtype=np.float32).view(np.uint32)
    r = (u + np.uint32(0x7FF) + ((u >> np.uint32(12)) & np.uint32(1))) & np.uint32(
        0xFFFFF000
    )
    return r.view(np.float32)


def build_program(T_main: int, H_: int, nwin: int):
    """Build the single-core SPMD Bass program.

    T_main: tokens per core (multiple of GW); H_: hidden dim (multiple of
    128); nwin: windows per core (= T_main // RATIO, multiple of WPG).
    """
    from contextlib import ExitStack

    import concourse.bacc as bacc
    import concourse.mybir as mybir
    import concourse.tile as tile

    f32 = mybir.dt.float32
    f32r = mybir.dt.bfloat16  # matmul stream dtype (name kept for brevity)
    AF = mybir.ActivationFunctionType
    AX = mybir.AxisListType

    d = HEAD_DIM
    r = RATIO
    NG = T_main // GW         # number of groups
    KT = H_ // 128            # k tiles
    C = 4 * d                 # 512 projection channels (kv_lo|kv_hi|g_lo|g_hi)
    NPAIR = NG // 2
    KKT = KT // 2
    PW = 2 * GW + r           # 1056 columns per pair load

    nc = bacc.Bacc("TRN2", target_bir_lowering=False, debug=False,
                   num_devices=N_CORES)
    # Pre-tiled on host in exact consumption order: each [128, 2, PW] block
    # is one fully-contiguous 1.08MB DMA (sequential HBM streaming).
    hTp = nc.dram_tensor("hTp", [NPAIR, KKT, 128, 2, PW], f32r,
                         kind="ExternalInput").ap()
    Wt = nc.dram_tensor("W", [KT, 128, C], f32r, kind="ExternalInput").ap()
    bias_lo = nc.dram_tensor("bias_lo", [d, GW], f32, kind="ExternalInput").ap()
    bias_lo0 = nc.dram_tensor("bias_lo0", [d, GW], f32, kind="ExternalInput").ap()
    bias_hi = nc.dram_tensor("bias_hi", [d, GW], f32, kind="ExternalInput").ap()
    cos_in = nc.dram_tensor("cos", [nwin, ROPE_DIM // 2], f32,
                            kind="ExternalInput").ap()
    sin_in = nc.dram_tensor("sin", [nwin, ROPE_DIM // 2], f32,
                            kind="ExternalInput").ap()
    ident = nc.dram_tensor("ident", [d, d], f32, kind="ExternalInput").ap()
    out = nc.dram_tensor("out", [nwin, d], f32, kind="ExternalOutput").ap()

    with tile.TileContext(nc) as tc, ExitStack() as ctx:
        wp = ctx.enter_context(tc.tile_pool(name="wp", bufs=1))
        hp = ctx.enter_context(tc.tile_pool(name="hp", bufs=1))
        pp = ctx.enter_context(tc.tile_pool(name="pp", bufs=1, space="PSUM"))
        sp = ctx.enter_context(tc.tile_pool(name="sp", bufs=2))
        smp = ctx.enter_context(tc.tile_pool(name="smp", bufs=2))
        cp = ctx.enter_context(tc.tile_pool(name="cp", bufs=1))

        # Stationary weights: one SBUF tile [128, KT, C].  Early k-tiles
        # (first half) ride the fast HWDGE queues interleaved with pair-0's
        # ht tiles by deadline; late k-tiles + biases + rope constants ride
        # the slow gpsimd (SWDGE) queue, whose deadlines are far out.
        # (ct: 0=kv_lo 1=kv_hi 2=g_lo 3=g_hi)
        w_all = wp.tile([128, KT, C], f32r, tag="w")

        def w_view(k, ct):
            return w_all[:, k, ct * d:(ct + 1) * d]

        def w_dma(eng, a, b):
            eng.dma_start(w_all[:, a:b, :],
                          Wt[a:b].rearrange("k p c -> p k c"))

        KHALF = KT // 2
        for a in range(KHALF, KT, 4):
            w_dma(nc.gpsimd, a, min(a + 4, KT))

        blo = cp.tile([d, GW], f32, tag="blo")
        blo0 = cp.tile([d, GW], f32, tag="blo0")
        bhi = cp.tile([d, GW], f32, tag="bhi")
        nc.gpsimd.dma_start(blo[:], bias_lo[:])
        nc.gpsimd.dma_start(blo0[:], bias_lo0[:])
        nc.gpsimd.dma_start(bhi[:], bias_hi[:])
        idt = cp.tile([d, d], f32, tag="idt")
        nc.gpsimd.dma_start(idt[:], ident[:])
        cosb = cp.tile([nwin, ROPE_DIM // 2], f32, tag="cosb")
        nc.gpsimd.dma_start(cosb[:], cos_in[:])
        sinb = cp.tile([nwin, ROPE_DIM // 2], f32, tag="sinb")
        nc.gpsimd.dma_start(sinb[:], sin_in[:])
        outsb = cp.tile([nwin, d], f32, tag="outsb")

        pooled = cp.tile([d, nwin], f32, tag="pooled")

        def pooling_group(g, ps, s_):
            # Softmax-gated pooling for one 512-token group; runs on
            # DVE/ACT/GpSimd in the shadow of the OTHER group's matmuls.
            # No max-subtraction: gate values are O(5), exp is safe, and
            # the -1e9 first-window fill underflows exp to exactly 0.
            # kv banks are copied out first so the PSUM slots free early.
            kvlo, kvhi, glo, ghi = ps
            t = {}
            t["kvlo"] = sp.tile([d, GW], f32, tag=f"kvlo_{s_}",
                                name=f"kvlo_{s_}{g}")
            nc.scalar.activation(t["kvlo"][:], kvlo[:], AF.Copy)
            t["kvhi"] = sp.tile([d, GW], f32, tag=f"kvhi_{s_}",
                                name=f"kvhi_{s_}{g}")
            nc.scalar.activation(t["kvhi"][:], kvhi[:], AF.Copy)
            t["tglo"] = sp.tile([d, GW], f32, tag=f"tglo_{s_}",
                                name=f"tglo_{s_}{g}")
            nc.vector.tensor_add(t["tglo"][:], glo[:],
                                 (blo0 if g == 0 else blo)[:])
            t["tghi"] = sp.tile([d, GW], f32, tag=f"tghi_{s_}",
                                name=f"tghi_{s_}{g}")
            nc.vector.tensor_add(t["tghi"][:], ghi[:], bhi[:])
            t["elo"] = sp.tile([d, GW], f32, tag=f"elo_{s_}",
                               name=f"elo_{s_}{g}")
            nc.scalar.activation(t["elo"][:], t["tglo"][:], AF.Exp)
            t["ehi"] = sp.tile([d, GW], f32, tag=f"ehi_{s_}",
                               name=f"ehi_{s_}{g}")
            nc.scalar.activation(t["ehi"][:], t["tghi"][:], AF.Exp)
            t["slo"] = smp.tile([d, WPG], f32, tag=f"slo_{s_}",
                                name=f"slo_{s_}{g}")
            nc.vector.reduce_sum(
                t["slo"][:], t["elo"][:].rearrange("p (w j) -> p w j", j=r),
                axis=AX.X)
            t["shi"] = smp.tile([d, WPG], f32, tag=f"shi_{s_}",
                                name=f"shi_{s_}{g}")
            nc.vector.reduce_sum(
                t["shi"][:], t["ehi"][:].rearrange("p (w j) -> p w j", j=r),
                axis=AX.X)
            t["plo"] = sp.tile([d, GW], f32, tag=f"plo_{s_}",
                               name=f"plo_{s_}{g}")
            nc.vector.tensor_mul(t["plo"][:], t["elo"][:], t["kvlo"][:])
            t["phi"] = sp.tile([d, GW], f32, tag=f"phi_{s_}",
                               name=f"phi_{s_}{g}")
            nc.vector.tensor_mul(t["phi"][:], t["ehi"][:], t["kvhi"][:])
            t["sall"] = smp.tile([d, WPG], f32, tag=f"sall_{s_}",
                                 name=f"sall_{s_}{g}")
            nc.gpsimd.tensor_add(t["sall"][:], t["slo"][:], t["shi"][:])
            t["nlo"] = smp.tile([d, WPG], f32, tag=f"nlo_{s_}",
                                name=f"nlo_{s_}{g}")
            nc.vector.reduce_sum(
                t["nlo"][:], t["plo"][:].rearrange("p (w j) -> p w j", j=r),
                axis=AX.X)
            t["nhi"] = smp.tile([d, WPG], f32, tag=f"nhi_{s_}",
                                name=f"nhi_{s_}{g}")
            nc.vector.reduce_sum(
                t["nhi"][:], t["phi"][:].rearrange("p (w j) -> p w j", j=r),
                axis=AX.X)
            t["rs"] = smp.tile([d, WPG], f32, tag=f"rs_{s_}",
                               name=f"rs_{s_}{g}")
            nc.vector.reciprocal(t["rs"][:], t["sall"][:])
            t["num"] = smp.tile([d, WPG], f32, tag=f"num_{s_}",
                                name=f"num_{s_}{g}")
            nc.vector.tensor_add(t["num"][:], t["nlo"][:], t["nhi"][:])
            nc.vector.tensor_mul(pooled[:, g * WPG:(g + 1) * WPG],
                                 t["num"][:], t["rs"][:])

        def issue_pair_dma(p, first=False):
            # One 540KB contiguous DMA per k-tile-pair, alternating between
            # the two HWDGE queues (sync, scalar).  For pair 0 the first
            # two tiles are split into per-k halves across both queues so
            # the first matmuls start ~2us after queue start; the bias
            # tiles ride sync afterwards (needed only at pair-0 drain).
            tiles = [hp.tile([128, 2, PW], f32r, tag=f"ht{kk}",
                             name=f"ht{kk}_{p}")
                     for kk in range(KKT)]
            if first and KKT >= 6:
                # Deadline-ordered interleave of early weight chunks and
                # pair-0 ht tiles on the two fast queues: the first pieces
                # are small (128-136KB) so the PE starts ~2.5us after the
                # queues do, and every chunk lands before its matmuls.
                HGW = GW + r
                w_dma(nc.sync, 0, 1)
                w_dma(nc.scalar, 1, 2)
                for j in range(2):
                    nc.sync.dma_start(tiles[0][:, j, 0:HGW],
                                      hTp[p, 0, :, j, 0:HGW])
                    nc.scalar.dma_start(tiles[0][:, j, HGW:PW],
                                        hTp[p, 0, :, j, HGW:PW])
                w_dma(nc.sync, 2, 4)
                w_dma(nc.scalar, 4, 6)
                nc.sync.dma_start(tiles[1][:, 0, :], hTp[p, 1, :, 0, :])
                nc.scalar.dma_start(tiles[1][:, 1, :], hTp[p, 1, :, 1, :])
                w_dma(nc.sync, 6, 8)
                w_dma(nc.scalar, 8, 10)
                nc.sync.dma_start(tiles[2][:, 0, :], hTp[p, 2, :, 0, :])
                nc.scalar.dma_start(tiles[2][:, 1, :], hTp[p, 2, :, 1, :])
                nc.sync.dma_start(tiles[3][:], hTp[p, 3])
                nc.scalar.dma_start(tiles[4][:], hTp[p, 4])
                w_dma(nc.sync, 10, 12)
                w_dma(nc.scalar, 12, 14)
                nc.sync.dma_start(tiles[5][:], hTp[p, 5])
                w_dma(nc.sync, 14, KHALF)
                for i in range(6, KKT):
                    (nc.scalar if i % 2 == 0 else nc.sync).dma_start(
                        tiles[i][:], hTp[p, i])
            elif first:
                HGW = GW + r
                w_dma(nc.sync, 0, 1)
                w_dma(nc.scalar, 1, KHALF)
                for j in range(2):
                    nc.sync.dma_start(tiles[0][:, j, 0:HGW],
                                      hTp[p, 0, :, j, 0:HGW])
                    nc.scalar.dma_start(tiles[0][:, j, HGW:PW],
                                        hTp[p, 0, :, j, HGW:PW])
                for i in range(1, KKT):
                    (nc.sync if i % 2 == 0 else nc.scalar).dma_start(
                        tiles[i][:], hTp[p, i])
            else:
                for kk in range(KKT):
                    (nc.sync if kk % 2 == 0 else nc.scalar).dma_start(
                        tiles[kk][:], hTp[p, kk])
            return tiles

        def mm_half(ps, tiles, base):
            # All KT k-tiles of one 512-token group: 128 back-to-back
            # matmuls accumulating into 4 PSUM banks.
            for kk in range(KKT):
                for j in range(2):
                    k = 2 * kk + j
                    ht_k = tiles[kk][:, j, :]
                    st, sp_ = (k == 0), (k == KT - 1)
                    for ct in range(4):
                        off = 0 if ct % 2 == 0 else r
                        nc.tensor.matmul(ps[ct][:], w_view(k, ct),
                                         ht_k[:, base + off:base + off + GW],
                                         start=st, stop=sp_)

        def mm_pair_interleaved(ps0, ps1, tiles):
            # Pair 0 only: both groups' matmuls per k-tile, so each ht tile
            # is consumed at half the rate -- matches the DMA stream while
            # it is still ramping (and sharing HBM with the weight load).
            for kk in range(KKT):
                for j in range(2):
                    k = 2 * kk + j
                    ht_k = tiles[kk][:, j, :]
                    st, sp_ = (k == 0), (k == KT - 1)
                    for base, ps in ((0, ps0), (GW, ps1)):
                        for ct in range(4):
                            off = base + (0 if ct % 2 == 0 else r)
                            nc.tensor.matmul(ps[ct][:], w_view(k, ct),
                                             ht_k[:, off:off + GW],
                                             start=st, stop=sp_)

        def mm_tail(ps, tiles, base, g, s_):
            # Last group: channel-major matmul order (g_lo, g_hi, kv_lo,
            # kv_hi) with the pooling chain emitted as each channel's
            # accumulation completes, so only a short mul+reduce tail runs
            # after the final matmul.  kv channels are read straight from
            # PSUM (no copy; the banks are not reused afterwards).
            kvlo, kvhi, glo, ghi = ps
            t = {}
            for ct in (2, 3, 0, 1):
                off = (0 if ct % 2 == 0 else r) + base
                for kk in range(KKT):
                    for j in range(2):
                        k = 2 * kk + j
                        nc.tensor.matmul(ps[ct][:], w_view(k, ct),
                                         tiles[kk][:, j, off:off + GW],
                                         start=(k == 0), stop=(k == KT - 1))
                if ct == 2:
                    t["tglo"] = sp.tile([d, GW], f32, tag=f"tglo_{s_}",
                                        name=f"tglo_{s_}{g}")
                    nc.vector.tensor_add(t["tglo"][:], glo[:], blo[:])
                    t["elo"] = sp.tile([d, GW], f32, tag=f"elo_{s_}",
                                       name=f"elo_{s_}{g}")
                    nc.scalar.activation(t["elo"][:], t["tglo"][:], AF.Exp)
                    t["slo"] = smp.tile([d, WPG], f32, tag=f"slo_{s_}",
                                        name=f"slo_{s_}{g}")
                    nc.vector.reduce_sum(
                        t["slo"][:],
                        t["elo"][:].rearrange("p (w j) -> p w j", j=r),
                        axis=AX.X)
                elif ct == 3:
                    t["tghi"] = sp.tile([d, GW], f32, tag=f"tghi_{s_}",
                                        name=f"tghi_{s_}{g}")
                    nc.vector.tensor_add(t["tghi"][:], ghi[:], bhi[:])
                    t["ehi"] = sp.tile([d, GW], f32, tag=f"ehi_{s_}",
                                       name=f"ehi_{s_}{g}")
                    nc.scalar.activation(t["ehi"][:], t["tghi"][:], AF.Exp)
                    t["shi"] = smp.tile([d, WPG], f32, tag=f"shi_{s_}",
                                        name=f"shi_{s_}{g}")
                    nc.vector.reduce_sum(
                        t["shi"][:],
                        t["ehi"][:].rearrange("p (w j) -> p w j", j=r),
                        axis=AX.X)
                    t["sall"] = smp.tile([d, WPG], f32, tag=f"sall_{s_}",
                                         name=f"sall_{s_}{g}")
                    nc.gpsimd.tensor_add(t["sall"][:], t["slo"][:],
                                         t["shi"][:])
                    t["rs"] = smp.tile([d, WPG], f32, tag=f"rs_{s_}",
                                       name=f"rs_{s_}{g}")
                    nc.vector.reciprocal(t["rs"][:], t["sall"][:])
                elif ct == 0:
                    t["plo"] = sp.tile([d, GW], f32, tag=f"plo_{s_}",
                                       name=f"plo_{s_}{g}")
                    nc.vector.tensor_mul(t["plo"][:], t["elo"][:], kvlo[:])
                    t["nlo"] = smp.tile([d, WPG], f32, tag=f"nlo_{s_}",
                                        name=f"nlo_{s_}{g}")
                    nc.vector.reduce_sum(
                        t["nlo"][:],
                        t["plo"][:].rearrange("p (w j) -> p w j", j=r),
                        axis=AX.X)
                else:
                    # Final chain after the very last matmul: split in
                    # column halves so the two sub-chains pipeline.
                    t["phi"] = sp.tile([d, GW], f32, tag=f"phi_{s_}",
                                       name=f"phi_{s_}{g}")
                    t["nhi"] = smp.tile([d, WPG], f32, tag=f"nhi_{s_}",
                                        name=f"nhi_{s_}{g}")
                    t["num"] = smp.tile([d, WPG], f32, tag=f"num_{s_}",
                                        name=f"num_{s_}{g}")
                    hg, hp_ = GW // 2, WPG // 2
                    for h in range(2):
                        cs = slice(h * hg, (h + 1) * hg)
                        ws = slice(h * hp_, (h + 1) * hp_)
                        nc.vector.tensor_mul(t["phi"][:, cs],
                                             t["ehi"][:, cs], kvhi[:, cs])
                        nc.vector.reduce_sum(
                            t["nhi"][:, ws],
                            t["phi"][:, cs].rearrange(
                                "p (w j) -> p w j", j=r),
                            axis=AX.X)
                        nc.gpsimd.tensor_add(t["num"][:, ws],
                                             t["nlo"][:, ws],
                                             t["nhi"][:, ws])
                        nc.vector.tensor_mul(
                            pooled[:, g * WPG + h * hp_:
                                   g * WPG + (h + 1) * hp_],
                            t["num"][:, ws], t["rs"][:, ws])

        PAIRW = 2 * WPG
        nope_w = d - ROPE_DIM
        hw_ = ROPE_DIM // 2

        def pair_epilogue(p):
            # Transpose pair p's 32 pooled columns and apply RoPE into
            # outsb.  Issued between the NEXT pair's two matmul phases, so
            # all inputs are long ready and the PE never waits; psum tag
            # kvlo_b is idle in exactly that slot.  The last pair's call is
            # the only one on the critical path.
            rows = slice(p * PAIRW, (p + 1) * PAIRW)
            ptr = pp.tile([PAIRW, d], f32, tag="kvlo_b", name=f"ptr{p}")
            nc.tensor.transpose(ptr[:], pooled[:, rows], idt[:])
            nc.vector.tensor_copy(outsb[rows, 0:nope_w], ptr[:, 0:nope_w])
            rp = ptr[:, nope_w:d].rearrange("p (a two) -> p a two", two=2)
            re_, ro_ = rp[:, :, 0], rp[:, :, 1]
            op = outsb[rows, nope_w:d].rearrange("p (a two) -> p a two",
                                                 two=2)
            oe_, oo_ = op[:, :, 0], op[:, :, 1]
            t1 = smp.tile([PAIRW, hw_], f32, tag="t1", name=f"t1_{p}")
            t2 = smp.tile([PAIRW, hw_], f32, tag="t2", name=f"t2_{p}")
            nc.vector.tensor_mul(t1[:], re_, cosb[rows, :])
            nc.vector.tensor_mul(t2[:], ro_, sinb[rows, :])
            nc.vector.tensor_sub(oe_, t1[:], t2[:])
            t3 = smp.tile([PAIRW, hw_], f32, tag="t3", name=f"t3_{p}")
            t4 = smp.tile([PAIRW, hw_], f32, tag="t4", name=f"t4_{p}")
            nc.vector.tensor_mul(t3[:], ro_, cosb[rows, :])
            nc.vector.tensor_mul(t4[:], re_, sinb[rows, :])
            nc.vector.tensor_add(oo_, t3[:], t4[:])

        # Pair 0 interleaved (DMA-ramp friendly); pairs 1+ two-phase
        # staggered: group A's 128 matmuls run while the previous group B's
        # pooling drains PSUM, and vice versa -- the PE never waits on a
        # pooling drain.  Next pair's DMAs are issued right after the last
        # matmuls reading the same ht buffers (correct WAR deps, transfers
        # overlap the current pair's compute).
        assert NG % 2 == 0 and KT % 2 == 0
        tiles = issue_pair_dma(0, first=True)
        for p in range(NPAIR):
            g0, g1 = 2 * p, 2 * p + 1
            last = (p == NPAIR - 1)
            ps0 = [pp.tile([d, GW], f32, tag=f"{t}_a", name=f"{t}_a{p}")
                   for t in ("kvlo", "kvhi", "glo", "ghi")]
            ps1 = [pp.tile([d, GW], f32, tag=f"{t}_b", name=f"{t}_b{p}")
                   for t in ("kvlo", "kvhi", "glo", "ghi")]
            if p == 0:
                mm_pair_interleaved(ps0, ps1, tiles)
                if not last:
                    tiles = issue_pair_dma(1)
                pooling_group(g0, ps0, "a")
                pooling_group(g1, ps1, "b")
                if last:
                    pair_epilogue(0)
                    nc.sync.dma_start(out[:], outsb[:])
            elif not last:
                mm_half(ps0, tiles, 0)
                pair_epilogue(p - 1)
                pooling_group(g0, ps0, "a")
                mm_half(ps1, tiles, GW)
                tiles = issue_pair_dma(p + 1)
                pooling_group(g1, ps1, "b")
            else:
                mm_half(ps0, tiles, 0)
                pair_epilogue(p - 1)
                rA = (NPAIR - 1) * PAIRW
                nc.sync.dma_start(out[0:rA], outsb[0:rA])
                pooling_group(g0, ps0, "a")
                mm_tail(ps1, tiles, GW, g1, "b")
                pair_epilogue(p)
                nc.sync.dma_start(out[rA:nwin], outsb[rA:nwin])

    nc.compile()
    return nc


def _host_inputs(hidden_states, w_kv, w_gate, position_bias,
                 T_main: int, nwin: int, n_cores: int):
    """Build per-core input maps (list of dicts) for the SPMD program."""
    d, r = HEAD_DIM, RATIO
    H_ = hidden_states.shape[2]
    n_total = nwin * n_cores // hidden_states.shape[0]  # windows per batch

    Wfull = np.concatenate([np.asarray(w_kv, np.float32),
                            np.asarray(w_gate, np.float32)], axis=1)
    Wr = np.ascontiguousarray(Wfull.astype(BF16).reshape(H_ // 128, 128, -1))

    biasT = np.ascontiguousarray(np.asarray(position_bias, np.float32).T)  # [d, 2r]
    bias_lo_t = np.ascontiguousarray(np.tile(biasT[:, :r], (1, WPG)))
    bias_hi_t = np.ascontiguousarray(np.tile(biasT[:, r:], (1, WPG)))
    bias_lo_g0 = bias_lo_t.copy()
    bias_lo_g0[:, :r] = NEG

    positions = np.arange(n_total, dtype=np.float32) * r
    inv_freq = 1.0 / (ROPE_THETA ** (
        np.arange(0, ROPE_DIM, 2, dtype=np.float32) / ROPE_DIM))
    freqs = positions[:, None] * inv_freq[None, :]         # [n_total, 32]
    cos = np.cos(freqs).astype(np.float32)
    sin = np.sin(freqs).astype(np.float32)
    ident = np.eye(d, dtype=np.float32)

    hs = np.asarray(hidden_states, np.float32)
    halves_per_batch = n_cores // hs.shape[0]
    NPAIR = T_main // (2 * GW)
    KKT = H_ // 256
    PW = 2 * GW + r
    in_maps = []
    for c in range(n_cores):
        b, hf = c // halves_per_batch, c % halves_per_batch
        start = hf * T_main
        chunk = np.empty((H_, T_main + r), BF16)
        chunk[:, r:] = hs[b, start:start + T_main].T
        if hf == 0:
            chunk[:, :r] = 0.0
        else:
            chunk[:, :r] = hs[b, start - r:start].T
        # Pre-tile into exact DMA consumption order:
        # hTp[pair, kk, p, j, c] = chunk[(2kk+j)*128 + p, pair*1024 + c]
        v = chunk.reshape(KKT, 2, 128, T_main + r)
        hTp = np.ascontiguousarray(
            np.stack([v[:, :, :, p0 * 2 * GW:p0 * 2 * GW + PW]
                      for p0 in range(NPAIR)], axis=0).transpose(0, 1, 3, 2, 4))
        w0 = hf * nwin
        in_maps.append({
            "hTp": hTp,
            "W": Wr,
            "bias_lo": bias_lo_t,
            "bias_lo0": bias_lo_g0 if hf == 0 else bias_lo_t,
            "bias_hi": bias_hi_t,
            "cos": np.ascontiguousarray(cos[w0:w0 + nwin]),
            "sin": np.ascontiguousarray(sin[w0:w0 + nwin]),
            "ident": ident,
        })
    return in_maps


def kernel(hidden_states, w_kv, w_gate, position_bias, _want_profile=False):
    """Full-input, full-output entry point.  Shards over 8 NeuronCores."""
    from concourse.bass_utils import run_bass_kernel_spmd

    hs = np.asarray(hidden_states, np.float32)
    B_, S_, H_ = hs.shape
    n = S_ // RATIO
    if "nc" not in _CACHE:
        _CACHE["nc"] = build_program(HALF, H_, NWIN_CORE)
    nc = _CACHE["nc"]

    in_maps = _host_inputs(hs, w_kv, w_gate, position_bias,
                           HALF, NWIN_CORE, N_CORES)
    kwargs = {}
    if _want_profile:
        import os
        import shutil

        shutil.rmtree("work/prof", ignore_errors=True)
        os.makedirs("work/prof", exist_ok=True)
        kwargs = {"trace": True, "tmpdir": os.path.abspath("work/prof")}
    res = run_bass_kernel_spmd(nc, in_maps, list(range(N_CORES)), **kwargs)

    out = np.empty((B_, n, HEAD_DIM), np.float32)
    halves_per_batch = N_CORES // B_
    for c in range(N_CORES):
        b, hf = c // halves_per_batch, c % halves_per_batch
        out[b, hf * NWIN_CORE:(hf + 1) * NWIN_CORE] = res.results[c]["out"]
    if _want_profile:
        return out, res
    return out

